# revision 4
# baseline (speedup 1.0000x reference)
"""NeRank loss kernel for 8 Trainium2 NeuronCores.

Strategy
--------
The dominant work is a shared-weight BiLSTM + fc applied to 12288 ragged
sequences (2048 positive + 2048*5 negative, each [L=64, I=300], lengths in
[1, 64]).  Everything after that (4 embedding gathers, elementwise adds, dot
products, log-sigmoid) reduces 12288x128 vectors to one scalar and is
negligible, so it runs on the host as the gather/unshard epilogue.

Device plan (SPMD, one program on 8 cores):
 * All 12288 sequences are sorted by length (ascending) and dealt round-robin
   to the 8 cores: global sorted rank i -> core i%8, column i//8.  Each core
   owns R=1536 columns, time-major transposed input xT[t, i, r].
 * Transposed LSTM layout: state h^T/c^T are [H=128 partitions, R free].
   Gates are computed as gates^T[4H, R] = Wih^T @ x_t^T + Whh^T @ h^T + b via
   TensorE matmuls (4 gate groups of 128 x (3 K-chunks of I + 1 K-chunk of H)).
 * Raggedness: since columns are sorted by length, the active columns at step
   t form a suffix [r0(t), R).  r0(t) is baked into the program (it is derived
   from the actual input lengths at kernel() time, min over cores so the same
   program is valid on all cores).  All DMA/matmul/vector work is restricted
   to the active suffix, so compute and memory scale with sum(len) rather than
   N*L.  Exact per-column semantics are enforced by a per-step mask
   (len > t) applied with copy_predicated on the h state only; the c state of
   finished columns may hold garbage, which is never observed.
 * Forward processes t = 0..T-1; backward processes t = T-1..0 (equivalent to
   the reference's reversed-sequence + reversed-mask formulation).  Both
   directions advance in the same program loop to keep all engines busy.
 * fc: out^T[D, R] = fc_W^T @ concat(h_f, h_b)^T + fc_b, then DMA out.

Host epilogue: un-permute embeddings, embedding-table gathers, dots,
log-sigmoid -> scalar loss.
"""

import os
from contextlib import ExitStack

import numpy as np

B, K, L, I, H, D = 2048, 5, 64, 300, 128, 128
N_CORES = 8
CELLW = 512  # R-tile width (one PSUM bank of fp32)

_PROGRAM_CACHE: dict = {}


# ---------------------------------------------------------------------------
# Host-side planning
# ---------------------------------------------------------------------------

def plan_sharding(lens: np.ndarray, n_cores: int):
    """Sort sequences by length, stripe across cores, compute active-suffix
    starts r0[t] (conservative min over cores: active columns at step t are
    exactly [r0_core(t), R) on every core with r0_core(t) >= r0[t], and
    columns < r0[t] are inactive on all cores)."""
    n = len(lens)
    assert n % n_cores == 0
    order = np.argsort(lens, kind="stable")
    lens_sorted = lens[order]
    T = int(lens_sorted[-1])
    R = n // n_cores
    r0 = [int(np.searchsorted(lens_sorted, t, side="right")) // n_cores
          for t in range(T)]
    return order, lens_sorted, T, R, r0


# ---------------------------------------------------------------------------
# Bass program builder
# ---------------------------------------------------------------------------

def build_program(T, R, r0_by_t, n_cores=N_CORES):
    import concourse.bass as bass  # noqa: F401
    import concourse.mybir as mybir
    import concourse.tile as tile
    from concourse import bacc

    f32 = mybir.dt.float32
    AF = mybir.ActivationFunctionType
    ALU = mybir.AluOpType

    nc = bacc.Bacc(
        "TRN2",
        target_bir_lowering=False,
        debug=False,
        enable_asserts=False,
        num_devices=n_cores,
    )

    xT_d = nc.dram_tensor("xT", [T, I, R], f32, kind="ExternalInput").ap()
    lenb_d = nc.dram_tensor("lenb", [128, R], f32, kind="ExternalInput").ap()
    wih_d = {
        "f": nc.dram_tensor("wih_f", [I, 4 * H], f32, kind="ExternalInput").ap(),
        "b": nc.dram_tensor("wih_b", [I, 4 * H], f32, kind="ExternalInput").ap(),
    }
    whh_d = {
        "f": nc.dram_tensor("whh_f", [H, 4 * H], f32, kind="ExternalInput").ap(),
        "b": nc.dram_tensor("whh_b", [H, 4 * H], f32, kind="ExternalInput").ap(),
    }
    b4_d = {
        "f": nc.dram_tensor("b4_f", [H, 4], f32, kind="ExternalInput").ap(),
        "b": nc.dram_tensor("b4_b", [H, 4], f32, kind="ExternalInput").ap(),
    }
    fcw_d = nc.dram_tensor("fcw", [2 * H, D], f32, kind="ExternalInput").ap()
    fcb_d = nc.dram_tensor("fcb", [D, 1], f32, kind="ExternalInput").ap()
    outT_d = nc.dram_tensor("outT", [D, R], f32, kind="ExternalOutput").ap()

    # I split into K-chunks of <=128 partitions
    KCH = [(o, min(128, I - o)) for o in range(0, I, 128)]
    cells = [(s, min(s + CELLW, R)) for s in range(0, R, CELLW)]

    with tile.TileContext(nc) as tc:
        with ExitStack() as ctx:
            consts = ctx.enter_context(tc.tile_pool(name="consts", bufs=1))
            states = ctx.enter_context(tc.tile_pool(name="states", bufs=1))
            xp = ctx.enter_context(tc.tile_pool(name="xp", bufs=4))
            gp = ctx.enter_context(tc.tile_pool(name="gates", bufs=3))
            tp = ctx.enter_context(tc.tile_pool(name="tmps", bufs=3))
            mp = ctx.enter_context(tc.tile_pool(name="masks", bufs=4))
            op = ctx.enter_context(tc.tile_pool(name="outs", bufs=2))
            pp = ctx.enter_context(tc.tile_pool(name="psum", bufs=2, space="PSUM"))

            # ---- constants ----
            wih = {}
            for d in ("f", "b"):
                w = consts.tile([128, len(KCH), 4 * H], f32, name=f"wih_{d}_sb")
                for ci, (o, kw) in enumerate(KCH):
                    nc.sync.dma_start(w[0:kw, ci, :], wih_d[d][o : o + kw, :])
                wih[d] = w
            whh = {}
            b4 = {}
            for d in ("f", "b"):
                w = consts.tile([128, 4 * H], f32, name=f"whh_{d}_sb")
                nc.sync.dma_start(w[:], whh_d[d])
                whh[d] = w
                bt = consts.tile([128, 4], f32, name=f"b4_{d}_sb")
                nc.sync.dma_start(bt[:], b4_d[d])
                b4[d] = bt
            fcw = consts.tile([128, 2, D], f32, name="fcw_sb")
            nc.sync.dma_start(fcw[:, 0, :], fcw_d[0:128, :])
            nc.sync.dma_start(fcw[:, 1, :], fcw_d[128:256, :])
            fcb = consts.tile([128, 1], f32, name="fcb_sb")
            nc.sync.dma_start(fcb[:], fcb_d)
            lenb = consts.tile([128, R], f32, name="lenb_sb")
            nc.sync.dma_start(lenb[:], lenb_d)

            # ---- states ----
            hs = {"f": [], "b": []}
            cs = {"f": [], "b": []}
            for d in ("f", "b"):
                for ri, (s, e) in enumerate(cells):
                    h = states.tile([128, e - s], f32, name=f"h_{d}_{ri}")
                    c = states.tile([128, e - s], f32, name=f"c_{d}_{ri}")
                    nc.vector.memset(h[:], 0.0)
                    nc.vector.memset(c[:], 0.0)
                    hs[d].append(h)
                    cs[d].append(c)

            GATE_FN = [AF.Sigmoid, AF.Sigmoid, AF.Tanh, AF.Sigmoid]

            # ---- recurrence ----
            for j in range(T):
                for d, t in (("f", j), ("b", T - 1 - j)):
                    a = r0_by_t[t]
                    if a >= R:
                        continue
                    for ri, (s, e) in enumerate(cells):
                        if a >= e:
                            continue
                        lo = max(s, (a // 32) * 32)
                        ow = lo - s
                        ce = e - s
                        hcell = hs[d][ri]
                        ccell = cs[d][ri]

                        m = mp.tile([128, CELLW], mybir.dt.uint8, tag="mask")
                        nc.vector.tensor_scalar(
                            m[:, ow:ce], lenb[:, lo:e], float(t), None, ALU.is_gt
                        )

                        xt = []
                        for ci, (o, kw) in enumerate(KCH):
                            x_ = xp.tile([128, CELLW], f32, tag=f"x{ci}")
                            nc.sync.dma_start(
                                x_[0:kw, ow:ce], xT_d[t, o : o + kw, lo:e]
                            )
                            xt.append(x_)

                        pses = []
                        for g in range(4):
                            ps = pp.tile([128, CELLW], f32, tag=f"ps{g}")
                            gsl = slice(g * H, (g + 1) * H)
                            for ci, (o, kw) in enumerate(KCH):
                                nc.tensor.matmul(
                                    ps[:, ow:ce],
                                    wih[d][0:kw, ci, gsl],
                                    xt[ci][0:kw, ow:ce],
                                    start=(ci == 0),
                                    stop=False,
                                )
                            nc.tensor.matmul(
                                ps[:, ow:ce],
                                whh[d][:, gsl],
                                hcell[:, ow:ce],
                                start=False,
                                stop=True,
                            )
                            pses.append(ps)

                        acts = []
                        for g in range(4):
                            at = gp.tile([128, CELLW], f32, tag=f"g{g}")
                            nc.scalar.activation(
                                at[:, ow:ce],
                                pses[g][:, ow:ce],
                                GATE_FN[g],
                                bias=b4[d][:, g : g + 1],
                            )
                            acts.append(at)
                        i_t, f_t, g_t, o_t = acts

                        t1 = tp.tile([128, CELLW], f32, tag="t1")
                        nc.vector.tensor_mul(
                            t1[:, ow:ce], f_t[:, ow:ce], ccell[:, ow:ce]
                        )
                        t2 = tp.tile([128, CELLW], f32, tag="t2")
                        nc.vector.tensor_mul(
                            t2[:, ow:ce], i_t[:, ow:ce], g_t[:, ow:ce]
                        )
                        if d == "f":
                            # forward: inactive columns are finished; their c
                            # is never observed again, so write unmasked.
                            nc.vector.tensor_add(
                                ccell[:, ow:ce], t1[:, ow:ce], t2[:, ow:ce]
                            )
                        else:
                            # backward: inactive columns have not started yet;
                            # c must stay 0 until the column activates.
                            cn = tp.tile([128, CELLW], f32, tag="cn")
                            nc.vector.tensor_add(
                                cn[:, ow:ce], t1[:, ow:ce], t2[:, ow:ce]
                            )
                            nc.vector.copy_predicated(
                                ccell[:, ow:ce], m[:, ow:ce], cn[:, ow:ce]
                            )
                        th = tp.tile([128, CELLW], f32, tag="th")
                        nc.scalar.activation(th[:, ow:ce], ccell[:, ow:ce], AF.Tanh)
                        hn = tp.tile([128, CELLW], f32, tag="hn")
                        nc.vector.tensor_mul(hn[:, ow:ce], o_t[:, ow:ce], th[:, ow:ce])
                        nc.vector.copy_predicated(
                            hcell[:, ow:ce], m[:, ow:ce], hn[:, ow:ce]
                        )

            # ---- fc ----
            for ri, (s, e) in enumerate(cells):
                ce = e - s
                ps = pp.tile([128, CELLW], f32, tag="ps0")
                nc.tensor.matmul(
                    ps[:, :ce], fcw[:, 0, :], hs["f"][ri][:, :ce],
                    start=True, stop=False,
                )
                nc.tensor.matmul(
                    ps[:, :ce], fcw[:, 1, :], hs["b"][ri][:, :ce],
                    start=False, stop=True,
                )
                ot = op.tile([128, CELLW], f32, tag="o")
                nc.scalar.activation(
                    ot[:, :ce], ps[:, :ce], AF.Identity, bias=fcb[:, 0:1]
                )
                nc.sync.dma_start(outT_d[:, s:e], ot[:, :ce])

    nc.compile()
    return nc


def get_program(T, R, r0_by_t, n_cores=N_CORES):
    key = (T, R, tuple(r0_by_t), n_cores)
    if key not in _PROGRAM_CACHE:
        _PROGRAM_CACHE.clear()
        _PROGRAM_CACHE[key] = build_program(T, R, r0_by_t, n_cores)
    return _PROGRAM_CACHE[key]


# ---------------------------------------------------------------------------
# Device embedding pass (sequences -> [N, D] embeddings)
# ---------------------------------------------------------------------------

def bilstm_embed_device(seqs, lens, Wih_f, Whh_f, b_f, Wih_b, Whh_b, b_b,
                        fc_W, fc_b, n_cores=N_CORES, trace=None):
    """seqs: [N, L, I] f32, lens: [N] int.  Returns ([N, D] f32, exec_ns)."""
    from concourse.bass_utils import run_bass_kernel_spmd

    n = seqs.shape[0]
    order, lens_sorted, T, R, r0 = plan_sharding(lens, n_cores)

    nc = get_program(T, R, r0, n_cores)

    xT_all = np.ascontiguousarray(seqs[:, :T, :].transpose(1, 2, 0))  # [T, I, N]
    xT_sorted = xT_all[:, :, order]
    del xT_all

    common = {
        "wih_f": np.ascontiguousarray(Wih_f, np.float32),
        "wih_b": np.ascontiguousarray(Wih_b, np.float32),
        "whh_f": np.ascontiguousarray(Whh_f, np.float32),
        "whh_b": np.ascontiguousarray(Whh_b, np.float32),
        "b4_f": np.ascontiguousarray(np.asarray(b_f, np.float32).reshape(4, H).T),
        "b4_b": np.ascontiguousarray(np.asarray(b_b, np.float32).reshape(4, H).T),
        "fcw": np.ascontiguousarray(fc_W, np.float32),
        "fcb": np.ascontiguousarray(np.asarray(fc_b, np.float32).reshape(D, 1)),
    }
    in_maps = []
    for c in range(n_cores):
        lens_c = lens_sorted[c::n_cores].astype(np.float32)
        in_map = dict(common)
        in_map["xT"] = np.ascontiguousarray(xT_sorted[:, :, c::n_cores])
        in_map["lenb"] = np.ascontiguousarray(
            np.broadcast_to(lens_c[None, :], (128, R))
        )
        in_maps.append(in_map)
    del xT_sorted

    if trace is None:
        trace = os.environ.get("NERANK_TRACE", "0") == "1"
    res = run_bass_kernel_spmd(
        nc, in_maps, core_ids=list(range(n_cores)), trace=trace,
        trace_cores=list(range(n_cores)) if trace else None,
    )
    outs = [r["outT"] for r in res.results]  # each [D, R]

    emb_sorted = np.stack(outs, axis=0).transpose(2, 0, 1).reshape(n, D)
    emb = np.empty((n, D), np.float32)
    emb[order] = emb_sorted
    return emb, res.exec_time_ns


# ---------------------------------------------------------------------------
# Full problem
# ---------------------------------------------------------------------------

def _log_sigmoid64(x):
    return -np.logaddexp(0.0, -np.float64(x))


def kernel(**inputs):
    qu_seq = np.asarray(inputs["qu_seq"], np.float32)
    qn_seq = np.asarray(inputs["qn_seq"], np.float32).reshape(B * K, L, I)
    lens = np.concatenate(
        [
            np.asarray(inputs["qu_len"]).astype(np.int64),
            np.asarray(inputs["qn_len"]).astype(np.int64).reshape(-1),
        ]
    )
    seqs = np.concatenate([qu_seq, qn_seq], axis=0)
    del qu_seq, qn_seq

    emb, exec_ns = bilstm_embed_device(
        seqs, lens,
        np.asarray(inputs["Wih_f"]), np.asarray(inputs["Whh_f"]),
        np.asarray(inputs["b_f"]),
        np.asarray(inputs["Wih_b"]), np.asarray(inputs["Whh_b"]),
        np.asarray(inputs["b_b"]),
        np.asarray(inputs["fc_W"]), np.asarray(inputs["fc_b"]),
    )
    del seqs
    kernel.last_exec_time_ns = exec_ns
    kernel._last_emb = emb
    globals()["_last_emb"] = emb

    embed_qu = emb[:B]
    neg_q = emb[B:].reshape(B, K, D)

    ru_emb = np.asarray(inputs["ru_emb"])
    au_emb = np.asarray(inputs["au_emb"])
    rv_emb = np.asarray(inputs["rv_emb"])
    av_emb = np.asarray(inputs["av_emb"])
    idx = {k: np.asarray(inputs[k]).astype(np.int64) for k in
           ("ru_idx", "au_idx", "rv_idx", "av_idx", "rn_idx", "an_idx")}

    embed_u = ru_emb[idx["ru_idx"]] + au_emb[idx["au_idx"]] + embed_qu
    embed_v = rv_emb[idx["rv_idx"]] + av_emb[idx["av_idx"]] + embed_qu
    score = np.sum(np.float64(embed_u) * np.float64(embed_v))

    neg_v = av_emb[idx["an_idx"]] + rv_emb[idx["rn_idx"]] + neg_q
    neg_score = np.einsum("bkd,bd->", np.float64(neg_v), np.float64(embed_u))

    loss = _log_sigmoid64(score) + _log_sigmoid64(-neg_score)
    return np.array(loss, dtype=np.float32)


kernel.last_exec_time_ns = None


# revision 8
# speedup vs baseline: 4.0361x; 4.0361x over previous
"""NeRank loss kernel for 8 Trainium2 NeuronCores.

Strategy
--------
The dominant work is a shared-weight BiLSTM + fc applied to 12288 ragged
sequences (2048 positive + 2048*5 negative, each [L=64, I=300], lengths in
[1, 64]).  Everything after that (4 embedding gathers, elementwise adds, dot
products, log-sigmoid) reduces 12288x128 vectors to one scalar and is
negligible, so it runs on the host as the gather/unshard epilogue.

Device plan (SPMD, one program on 8 cores):
 * All 12288 sequences are sorted by length (ascending) and dealt round-robin
   to the 8 cores: global sorted rank i -> core i%8, column i//8.  Each core
   owns R=1536 columns, time-major transposed input xT[t, i, r].
 * Transposed LSTM layout: state h^T/c^T are [H=128 partitions, R free].
   Gates are computed as gates^T[4H, R] = Wih^T @ x_t^T + Whh^T @ h^T + b via
   TensorE matmuls (4 gate groups of 128 x (3 K-chunks of I + 1 K-chunk of H)).
 * Raggedness: since columns are sorted by length, the active columns at step
   t form a suffix [r0(t), R).  r0(t) is baked into the program (it is derived
   from the actual input lengths at kernel() time, min over cores so the same
   program is valid on all cores).  All DMA/matmul/vector work is restricted
   to the active suffix, so compute and memory scale with sum(len) rather than
   N*L.  Exact per-column semantics are enforced by a per-step mask
   (len > t) applied with copy_predicated on the h state only; the c state of
   finished columns may hold garbage, which is never observed.
 * Forward processes t = 0..T-1; backward processes t = T-1..0 (equivalent to
   the reference's reversed-sequence + reversed-mask formulation).  Both
   directions advance in the same program loop to keep all engines busy.
 * fc: out^T[D, R] = fc_W^T @ concat(h_f, h_b)^T + fc_b, then DMA out.

Host epilogue: un-permute embeddings, embedding-table gathers, dots,
log-sigmoid -> scalar loss.
"""

import os
from contextlib import ExitStack

import numpy as np

B, K, L, I, H, D = 2048, 5, 64, 300, 128, 128
N_CORES = 8
CELLW = 512  # R-tile width (one PSUM bank of fp32)

_PROGRAM_CACHE: dict = {}


# ---------------------------------------------------------------------------
# Host-side planning
# ---------------------------------------------------------------------------

def plan_sharding(lens: np.ndarray, n_cores: int):
    """Sort sequences by length, stripe across cores, compute active-suffix
    starts r0[t] (conservative min over cores: active columns at step t are
    exactly [r0_core(t), R) on every core with r0_core(t) >= r0[t], and
    columns < r0[t] are inactive on all cores)."""
    n = len(lens)
    assert n % n_cores == 0
    order = np.argsort(lens, kind="stable")
    lens_sorted = lens[order]
    T = int(lens_sorted[-1])
    R = n // n_cores
    r0 = [int(np.searchsorted(lens_sorted, t, side="right")) // n_cores
          for t in range(T)]
    return order, lens_sorted, T, R, r0


# ---------------------------------------------------------------------------
# Bass program builder
# ---------------------------------------------------------------------------

def build_program(T, R, r0_by_t, n_cores=N_CORES, mm_bf16=True):
    import concourse.bass as bass  # noqa: F401
    import concourse.mybir as mybir
    import concourse.tile as tile
    from concourse import bacc

    f32 = mybir.dt.float32
    dtm = mybir.dt.bfloat16 if mm_bf16 else f32  # matmul operand dtype
    AF = mybir.ActivationFunctionType
    ALU = mybir.AluOpType

    nc = bacc.Bacc(
        "TRN2",
        target_bir_lowering=False,
        debug=False,
        enable_asserts=False,
        num_devices=n_cores,
    )

    xT_d = nc.dram_tensor("xT", [T, I, R], dtm, kind="ExternalInput").ap()
    lenb_d = nc.dram_tensor("lenb", [128, R], f32, kind="ExternalInput").ap()
    wih_d = {
        "f": nc.dram_tensor("wih_f", [I, 4 * H], dtm, kind="ExternalInput").ap(),
        "b": nc.dram_tensor("wih_b", [I, 4 * H], dtm, kind="ExternalInput").ap(),
    }
    whh_d = {
        "f": nc.dram_tensor("whh_f", [H, 4 * H], dtm, kind="ExternalInput").ap(),
        "b": nc.dram_tensor("whh_b", [H, 4 * H], dtm, kind="ExternalInput").ap(),
    }
    b4_d = {
        "f": nc.dram_tensor("b4_f", [H, 4], f32, kind="ExternalInput").ap(),
        "b": nc.dram_tensor("b4_b", [H, 4], f32, kind="ExternalInput").ap(),
    }
    fcw_d = nc.dram_tensor("fcw", [2 * H, D], dtm, kind="ExternalInput").ap()
    fcb_d = nc.dram_tensor("fcb", [D, 1], f32, kind="ExternalInput").ap()
    outT_d = nc.dram_tensor("outT", [D, R], f32, kind="ExternalOutput").ap()

    # I split into K-chunks of <=128 partitions
    KCH = [(o, min(128, I - o)) for o in range(0, I, 128)]
    cells = [(s, min(s + CELLW, R)) for s in range(0, R, CELLW)]

    with tile.TileContext(nc) as tc:
        with ExitStack() as ctx:
            consts = ctx.enter_context(tc.tile_pool(name="consts", bufs=1))
            states = ctx.enter_context(tc.tile_pool(name="states", bufs=1))
            xp = ctx.enter_context(tc.tile_pool(name="xp", bufs=4))
            gp = ctx.enter_context(tc.tile_pool(name="gates", bufs=3))
            tp = ctx.enter_context(tc.tile_pool(name="tmps", bufs=3))
            mp = ctx.enter_context(tc.tile_pool(name="masks", bufs=4))
            op = ctx.enter_context(tc.tile_pool(name="outs", bufs=2))
            pp = ctx.enter_context(tc.tile_pool(name="psum", bufs=2, space="PSUM"))

            # ---- constants ----
            wih = {}
            for d in ("f", "b"):
                w = consts.tile([128, len(KCH), 4 * H], dtm, name=f"wih_{d}_sb")
                for ci, (o, kw) in enumerate(KCH):
                    nc.sync.dma_start(w[0:kw, ci, :], wih_d[d][o : o + kw, :])
                wih[d] = w
            whh = {}
            b4 = {}
            for d in ("f", "b"):
                w = consts.tile([128, 4 * H], dtm, name=f"whh_{d}_sb")
                nc.sync.dma_start(w[:], whh_d[d])
                whh[d] = w
                bt = consts.tile([128, 4], f32, name=f"b4_{d}_sb")
                nc.sync.dma_start(bt[:], b4_d[d])
                b4[d] = bt
            fcw = consts.tile([128, 2, D], dtm, name="fcw_sb")
            nc.sync.dma_start(fcw[:, 0, :], fcw_d[0:128, :])
            nc.sync.dma_start(fcw[:, 1, :], fcw_d[128:256, :])
            fcb = consts.tile([128, 1], f32, name="fcb_sb")
            nc.sync.dma_start(fcb[:], fcb_d)
            lenb = consts.tile([128, R], f32, name="lenb_sb")
            nc.sync.dma_start(lenb[:], lenb_d)

            # ---- states ----
            hs = {"f": [], "b": []}
            cs = {"f": [], "b": []}
            for d in ("f", "b"):
                for ri, (s, e) in enumerate(cells):
                    h = states.tile([128, e - s], dtm, name=f"h_{d}_{ri}")
                    c = states.tile([128, e - s], f32, name=f"c_{d}_{ri}")
                    nc.vector.memset(h[:], 0.0)
                    nc.vector.memset(c[:], 0.0)
                    hs[d].append(h)
                    cs[d].append(c)

            GATE_FN = [AF.Sigmoid, AF.Sigmoid, AF.Tanh, AF.Sigmoid]

            # ---- recurrence ----
            for j in range(T):
                for d, t in (("f", j), ("b", T - 1 - j)):
                    a = r0_by_t[t]
                    if a >= R:
                        continue
                    for ri, (s, e) in enumerate(cells):
                        if a >= e:
                            continue
                        lo = max(s, (a // 32) * 32)
                        ow = lo - s
                        ce = e - s
                        hcell = hs[d][ri]
                        ccell = cs[d][ri]

                        m = mp.tile([128, CELLW], mybir.dt.uint8, tag="mask")
                        nc.vector.tensor_scalar(
                            m[:, ow:ce], lenb[:, lo:e], float(t), None, ALU.is_gt
                        )

                        xt = []
                        for ci, (o, kw) in enumerate(KCH):
                            x_ = xp.tile([128, CELLW], dtm, tag=f"x{ci}")
                            nc.sync.dma_start(
                                x_[0:kw, ow:ce], xT_d[t, o : o + kw, lo:e]
                            )
                            xt.append(x_)

                        pses = []
                        for g in range(4):
                            ps = pp.tile([128, CELLW], f32, tag=f"ps{g}")
                            gsl = slice(g * H, (g + 1) * H)
                            for ci, (o, kw) in enumerate(KCH):
                                nc.tensor.matmul(
                                    ps[:, ow:ce],
                                    wih[d][0:kw, ci, gsl],
                                    xt[ci][0:kw, ow:ce],
                                    start=(ci == 0),
                                    stop=False,
                                )
                            nc.tensor.matmul(
                                ps[:, ow:ce],
                                whh[d][:, gsl],
                                hcell[:, ow:ce],
                                start=False,
                                stop=True,
                            )
                            pses.append(ps)

                        acts = []
                        for g in range(4):
                            at = gp.tile([128, CELLW], f32, tag=f"g{g}")
                            nc.scalar.activation(
                                at[:, ow:ce],
                                pses[g][:, ow:ce],
                                GATE_FN[g],
                                bias=b4[d][:, g : g + 1],
                            )
                            acts.append(at)
                        i_t, f_t, g_t, o_t = acts

                        t1 = tp.tile([128, CELLW], f32, tag="t1")
                        nc.vector.tensor_mul(
                            t1[:, ow:ce], f_t[:, ow:ce], ccell[:, ow:ce]
                        )
                        t2 = tp.tile([128, CELLW], f32, tag="t2")
                        nc.vector.tensor_mul(
                            t2[:, ow:ce], i_t[:, ow:ce], g_t[:, ow:ce]
                        )
                        if d == "f":
                            # forward: inactive columns are finished; their c
                            # is never observed again, so write unmasked.
                            nc.vector.tensor_add(
                                ccell[:, ow:ce], t1[:, ow:ce], t2[:, ow:ce]
                            )
                        else:
                            # backward: inactive columns have not started yet;
                            # c must stay 0 until the column activates.
                            cn = tp.tile([128, CELLW], f32, tag="cn")
                            nc.vector.tensor_add(
                                cn[:, ow:ce], t1[:, ow:ce], t2[:, ow:ce]
                            )
                            nc.vector.copy_predicated(
                                ccell[:, ow:ce], m[:, ow:ce], cn[:, ow:ce]
                            )
                        th = tp.tile([128, CELLW], f32, tag="th")
                        nc.scalar.activation(th[:, ow:ce], ccell[:, ow:ce], AF.Tanh)
                        hn = tp.tile([128, CELLW], dtm, tag="hn")
                        nc.vector.tensor_mul(hn[:, ow:ce], o_t[:, ow:ce], th[:, ow:ce])
                        nc.vector.copy_predicated(
                            hcell[:, ow:ce], m[:, ow:ce], hn[:, ow:ce]
                        )

            # ---- fc ----
            for ri, (s, e) in enumerate(cells):
                ce = e - s
                ps = pp.tile([128, CELLW], f32, tag="ps0")
                nc.tensor.matmul(
                    ps[:, :ce], fcw[:, 0, :], hs["f"][ri][:, :ce],
                    start=True, stop=False,
                )
                nc.tensor.matmul(
                    ps[:, :ce], fcw[:, 1, :], hs["b"][ri][:, :ce],
                    start=False, stop=True,
                )
                ot = op.tile([128, CELLW], f32, tag="o")
                nc.scalar.activation(
                    ot[:, :ce], ps[:, :ce], AF.Identity, bias=fcb[:, 0:1]
                )
                nc.sync.dma_start(outT_d[:, s:e], ot[:, :ce])

    nc.compile()
    return nc


def get_program(T, R, r0_by_t, n_cores=N_CORES, mm_bf16=True):
    key = (T, R, tuple(r0_by_t), n_cores, mm_bf16)
    if key not in _PROGRAM_CACHE:
        _PROGRAM_CACHE.clear()
        _PROGRAM_CACHE[key] = build_program(T, R, r0_by_t, n_cores, mm_bf16)
    return _PROGRAM_CACHE[key]


# ---------------------------------------------------------------------------
# Device embedding pass (sequences -> [N, D] embeddings)
# ---------------------------------------------------------------------------

MM_BF16 = os.environ.get("NERANK_FP32", "0") != "1"


def bilstm_embed_device(seqs, lens, Wih_f, Whh_f, b_f, Wih_b, Whh_b, b_b,
                        fc_W, fc_b, n_cores=N_CORES, trace=None):
    """seqs: [N, L, I] f32, lens: [N] int.  Returns ([N, D] f32, exec_ns)."""
    import ml_dtypes

    from concourse.bass_utils import run_bass_kernel_spmd

    n = seqs.shape[0]
    order, lens_sorted, T, R, r0 = plan_sharding(lens, n_cores)

    nc = get_program(T, R, r0, n_cores, MM_BF16)
    dtm_np = ml_dtypes.bfloat16 if MM_BF16 else np.float32

    xT_all = np.ascontiguousarray(
        seqs[:, :T, :].transpose(1, 2, 0).astype(dtm_np)
    )  # [T, I, N]
    xT_sorted = xT_all[:, :, order]
    del xT_all

    common = {
        "wih_f": np.ascontiguousarray(np.asarray(Wih_f).astype(dtm_np)),
        "wih_b": np.ascontiguousarray(np.asarray(Wih_b).astype(dtm_np)),
        "whh_f": np.ascontiguousarray(np.asarray(Whh_f).astype(dtm_np)),
        "whh_b": np.ascontiguousarray(np.asarray(Whh_b).astype(dtm_np)),
        "b4_f": np.ascontiguousarray(np.asarray(b_f, np.float32).reshape(4, H).T),
        "b4_b": np.ascontiguousarray(np.asarray(b_b, np.float32).reshape(4, H).T),
        "fcw": np.ascontiguousarray(np.asarray(fc_W).astype(dtm_np)),
        "fcb": np.ascontiguousarray(np.asarray(fc_b, np.float32).reshape(D, 1)),
    }
    in_maps = []
    for c in range(n_cores):
        lens_c = lens_sorted[c::n_cores].astype(np.float32)
        in_map = dict(common)
        in_map["xT"] = np.ascontiguousarray(xT_sorted[:, :, c::n_cores])
        in_map["lenb"] = np.ascontiguousarray(
            np.broadcast_to(lens_c[None, :], (128, R))
        )
        in_maps.append(in_map)
    del xT_sorted

    if trace is None:
        trace = os.environ.get("NERANK_TRACE", "0") == "1"
    res = run_bass_kernel_spmd(
        nc, in_maps, core_ids=list(range(n_cores)), trace=trace,
        trace_cores=list(range(n_cores)) if trace else None,
    )
    outs = [r["outT"] for r in res.results]  # each [D, R]

    emb_sorted = np.stack(outs, axis=0).transpose(2, 0, 1).reshape(n, D)
    emb = np.empty((n, D), np.float32)
    emb[order] = emb_sorted
    return emb, res.exec_time_ns


# ---------------------------------------------------------------------------
# Full problem
# ---------------------------------------------------------------------------

def _log_sigmoid64(x):
    return -np.logaddexp(0.0, -np.float64(x))


def kernel(**inputs):
    qu_seq = np.asarray(inputs["qu_seq"], np.float32)
    qn_seq = np.asarray(inputs["qn_seq"], np.float32).reshape(B * K, L, I)
    lens = np.concatenate(
        [
            np.asarray(inputs["qu_len"]).astype(np.int64),
            np.asarray(inputs["qn_len"]).astype(np.int64).reshape(-1),
        ]
    )
    seqs = np.concatenate([qu_seq, qn_seq], axis=0)
    del qu_seq, qn_seq

    emb, exec_ns = bilstm_embed_device(
        seqs, lens,
        np.asarray(inputs["Wih_f"]), np.asarray(inputs["Whh_f"]),
        np.asarray(inputs["b_f"]),
        np.asarray(inputs["Wih_b"]), np.asarray(inputs["Whh_b"]),
        np.asarray(inputs["b_b"]),
        np.asarray(inputs["fc_W"]), np.asarray(inputs["fc_b"]),
    )
    del seqs
    kernel.last_exec_time_ns = exec_ns
    kernel._last_emb = emb
    globals()["_last_emb"] = emb

    embed_qu = emb[:B]
    neg_q = emb[B:].reshape(B, K, D)

    ru_emb = np.asarray(inputs["ru_emb"])
    au_emb = np.asarray(inputs["au_emb"])
    rv_emb = np.asarray(inputs["rv_emb"])
    av_emb = np.asarray(inputs["av_emb"])
    idx = {k: np.asarray(inputs[k]).astype(np.int64) for k in
           ("ru_idx", "au_idx", "rv_idx", "av_idx", "rn_idx", "an_idx")}

    embed_u = ru_emb[idx["ru_idx"]] + au_emb[idx["au_idx"]] + embed_qu
    embed_v = rv_emb[idx["rv_idx"]] + av_emb[idx["av_idx"]] + embed_qu
    score = np.sum(np.float64(embed_u) * np.float64(embed_v))

    neg_v = av_emb[idx["an_idx"]] + rv_emb[idx["rn_idx"]] + neg_q
    neg_score = np.einsum("bkd,bd->", np.float64(neg_v), np.float64(embed_u))

    loss = _log_sigmoid64(score) + _log_sigmoid64(-neg_score)
    return np.array(loss, dtype=np.float32)


kernel.last_exec_time_ns = None


# revision 11
# speedup vs baseline: 4.2186x; 1.0452x over previous
"""NeRank loss kernel for 8 Trainium2 NeuronCores.

Strategy
--------
The dominant work is a shared-weight BiLSTM + fc applied to 12288 ragged
sequences (2048 positive + 2048*5 negative, each [L=64, I=300], lengths in
[1, 64]).  Everything after that (4 embedding gathers, elementwise adds, dot
products, log-sigmoid) reduces 12288x128 vectors to one scalar and is
negligible, so it runs on the host as the gather/unshard epilogue.

Device plan (SPMD, one program on 8 cores):
 * All 12288 sequences are sorted by length (ascending) and dealt round-robin
   to the 8 cores: global sorted rank i -> core i%8, column i//8.  Each core
   owns R=1536 columns, time-major transposed input xT[t, i, r].
 * Transposed LSTM layout: state h^T/c^T are [H=128 partitions, R free].
   Gates are computed as gates^T[4H, R] = Wih^T @ x_t^T + Whh^T @ h^T + b via
   TensorE matmuls (4 gate groups of 128 x (3 K-chunks of I + 1 K-chunk of H)).
 * Raggedness: since columns are sorted by length, the active columns at step
   t form a suffix [r0(t), R).  r0(t) is baked into the program (it is derived
   from the actual input lengths at kernel() time, min over cores so the same
   program is valid on all cores).  All DMA/matmul/vector work is restricted
   to the active suffix, so compute and memory scale with sum(len) rather than
   N*L.  Exact per-column semantics are enforced by a per-step mask
   (len > t) applied with copy_predicated on the h state only; the c state of
   finished columns may hold garbage, which is never observed.
 * Forward processes t = 0..T-1; backward processes t = T-1..0 (equivalent to
   the reference's reversed-sequence + reversed-mask formulation).  Both
   directions advance in the same program loop to keep all engines busy.
 * fc: out^T[D, R] = fc_W^T @ concat(h_f, h_b)^T + fc_b, then DMA out.

Host epilogue: un-permute embeddings, embedding-table gathers, dots,
log-sigmoid -> scalar loss.
"""

import os
from contextlib import ExitStack

import numpy as np

B, K, L, I, H, D = 2048, 5, 64, 300, 128, 128
N_CORES = 8
CELLW = 512  # R-tile width (one PSUM bank of fp32)

_PROGRAM_CACHE: dict = {}


# ---------------------------------------------------------------------------
# Host-side planning
# ---------------------------------------------------------------------------

def plan_sharding(lens: np.ndarray, n_cores: int):
    """Sort sequences by length, stripe across cores, compute active-suffix
    starts r0[t] (conservative min over cores: active columns at step t are
    exactly [r0_core(t), R) on every core with r0_core(t) >= r0[t], and
    columns < r0[t] are inactive on all cores)."""
    n = len(lens)
    assert n % n_cores == 0
    order = np.argsort(lens, kind="stable")
    lens_sorted = lens[order]
    T = int(lens_sorted[-1])
    R = n // n_cores
    r0 = [int(np.searchsorted(lens_sorted, t, side="right")) // n_cores
          for t in range(T)]
    return order, lens_sorted, T, R, r0


# ---------------------------------------------------------------------------
# Bass program builder
# ---------------------------------------------------------------------------

def build_program(T, R, r0_by_t, n_cores=N_CORES, mm_bf16=True):
    import concourse.bass as bass  # noqa: F401
    import concourse.mybir as mybir
    import concourse.tile as tile
    from concourse import bacc

    f32 = mybir.dt.float32
    dtm = mybir.dt.bfloat16 if mm_bf16 else f32  # matmul operand dtype
    AF = mybir.ActivationFunctionType
    ALU = mybir.AluOpType

    nc = bacc.Bacc(
        "TRN2",
        target_bir_lowering=False,
        debug=False,
        enable_asserts=False,
        num_devices=n_cores,
    )

    IA = I + 1  # x augmented with a constant-1 row; Wih row I holds the bias
    xT_d = nc.dram_tensor("xT", [T, IA, R], dtm, kind="ExternalInput").ap()
    lenb_d = nc.dram_tensor("lenb", [128, R], f32, kind="ExternalInput").ap()
    wih_d = {
        "f": nc.dram_tensor("wih_f", [IA, 4 * H], dtm, kind="ExternalInput").ap(),
        "b": nc.dram_tensor("wih_b", [IA, 4 * H], dtm, kind="ExternalInput").ap(),
    }
    whh_d = {
        "f": nc.dram_tensor("whh_f", [H, 4 * H], dtm, kind="ExternalInput").ap(),
        "b": nc.dram_tensor("whh_b", [H, 4 * H], dtm, kind="ExternalInput").ap(),
    }
    fcw_d = nc.dram_tensor("fcw", [2 * H, D], dtm, kind="ExternalInput").ap()
    fcb_d = nc.dram_tensor("fcb", [D, 1], f32, kind="ExternalInput").ap()
    outT_d = nc.dram_tensor("outT", [D, R], f32, kind="ExternalOutput").ap()

    # I+1 split into K-chunks of <=128 partitions
    KCH = [(o, min(128, IA - o)) for o in range(0, IA, 128)]
    cells = [(s, min(s + CELLW, R)) for s in range(0, R, CELLW)]

    with tile.TileContext(nc) as tc:
        with ExitStack() as ctx:
            consts = ctx.enter_context(tc.tile_pool(name="consts", bufs=1))
            states = ctx.enter_context(tc.tile_pool(name="states", bufs=1))
            xp = ctx.enter_context(tc.tile_pool(name="xp", bufs=4))
            gp = ctx.enter_context(tc.tile_pool(name="gates", bufs=3))
            tp = ctx.enter_context(tc.tile_pool(name="tmps", bufs=3))
            mp = ctx.enter_context(tc.tile_pool(name="masks", bufs=4))
            op = ctx.enter_context(tc.tile_pool(name="outs", bufs=2))
            pp = ctx.enter_context(tc.tile_pool(name="psum", bufs=2, space="PSUM"))

            # ---- constants ----
            wih = {}
            for d in ("f", "b"):
                w = consts.tile([128, len(KCH), 4 * H], dtm, name=f"wih_{d}_sb")
                for ci, (o, kw) in enumerate(KCH):
                    nc.sync.dma_start(w[0:kw, ci, :], wih_d[d][o : o + kw, :])
                wih[d] = w
            whh = {}
            for d in ("f", "b"):
                w = consts.tile([128, 4 * H], dtm, name=f"whh_{d}_sb")
                nc.sync.dma_start(w[:], whh_d[d])
                whh[d] = w
            fcw = consts.tile([128, 2, D], dtm, name="fcw_sb")
            nc.sync.dma_start(fcw[:, 0, :], fcw_d[0:128, :])
            nc.sync.dma_start(fcw[:, 1, :], fcw_d[128:256, :])
            fcb = consts.tile([128, 1], f32, name="fcb_sb")
            nc.sync.dma_start(fcb[:], fcb_d)
            lenb = consts.tile([128, R], f32, name="lenb_sb")
            nc.sync.dma_start(lenb[:], lenb_d)

            # ---- states ----
            hs = {"f": [], "b": []}
            cs = {"f": [], "b": []}
            for d in ("f", "b"):
                for ri, (s, e) in enumerate(cells):
                    h = states.tile([128, e - s], dtm, name=f"h_{d}_{ri}")
                    c = states.tile([128, e - s], f32, name=f"c_{d}_{ri}")
                    nc.vector.memset(h[:], 0.0)
                    nc.vector.memset(c[:], 0.0)
                    hs[d].append(h)
                    cs[d].append(c)

            # gate column order (host-permuted): 0=i, 1=f, 2=o, 3=g
            # ---- recurrence ----
            for j in range(T):
                for d, t in (("f", j), ("b", T - 1 - j)):
                    a = r0_by_t[t]
                    if a >= R:
                        continue
                    for ri, (s, e) in enumerate(cells):
                        if a >= e:
                            continue
                        lo = max(s, (a // 32) * 32)
                        bk = max(s, min(a + 1, e))  # bulk start: active on ALL cores
                        ow = lo - s
                        bw = bk - s
                        ce = e - s
                        hcell = hs[d][ri]
                        ccell = cs[d][ri]
                        has_strip = bw > ow

                        xt = []
                        for ci, (o, kw) in enumerate(KCH):
                            x_ = xp.tile([128, CELLW], dtm, tag=f"x{ci}")
                            nc.sync.dma_start(
                                x_[0:kw, ow:ce], xT_d[t, o : o + kw, lo:e]
                            )
                            xt.append(x_)

                        ps = pp.tile([128, 4, CELLW], f32, tag="ps")
                        for ci, (o, kw) in enumerate(KCH):
                            for g in range(4):
                                nc.tensor.matmul(
                                    ps[:, g, ow:ce],
                                    wih[d][0:kw, ci, g * H : (g + 1) * H],
                                    xt[ci][0:kw, ow:ce],
                                    start=(ci == 0),
                                    stop=False,
                                )
                        for g in range(4):
                            nc.tensor.matmul(
                                ps[:, g, ow:ce],
                                whh[d][:, g * H : (g + 1) * H],
                                hcell[:, ow:ce],
                                start=False,
                                stop=True,
                            )

                        # activations: one sigmoid over (i,f,o), one tanh for g
                        sg = gp.tile([128, 3, CELLW], f32, tag="sg")
                        nc.scalar.activation(
                            sg[:, :, ow:ce], ps[:, 0:3, ow:ce], AF.Sigmoid
                        )
                        tg = gp.tile([128, CELLW], f32, tag="tg")
                        nc.scalar.activation(tg[:, ow:ce], ps[:, 3, ow:ce], AF.Tanh)

                        i_t = sg[:, 0, :]
                        f_t = sg[:, 1, :]
                        o_t = sg[:, 2, :]

                        t1 = tp.tile([128, CELLW], f32, tag="t1")
                        nc.vector.tensor_mul(
                            t1[:, ow:ce], f_t[:, ow:ce], ccell[:, ow:ce]
                        )
                        t2 = tp.tile([128, CELLW], f32, tag="t2")
                        nc.vector.tensor_mul(
                            t2[:, ow:ce], i_t[:, ow:ce], tg[:, ow:ce]
                        )

                        # c update: bulk columns (active on every core) in place
                        if d == "f":
                            # forward: inactive columns are finished; garbage c
                            # in the strip is never observed -> no predication.
                            nc.vector.tensor_add(
                                ccell[:, ow:ce], t1[:, ow:ce], t2[:, ow:ce]
                            )
                        else:
                            if ce > bw:
                                nc.vector.tensor_add(
                                    ccell[:, bw:ce], t1[:, bw:ce], t2[:, bw:ce]
                                )
                            if has_strip:
                                m = mp.tile(
                                    [128, 64], mybir.dt.uint8, tag="mask"
                                )
                                nc.vector.tensor_scalar(
                                    m[:, : bw - ow], lenb[:, lo:bk],
                                    float(t), None, ALU.is_gt,
                                )
                                cn = tp.tile([128, 64], f32, tag="cn")
                                nc.vector.tensor_add(
                                    cn[:, : bw - ow], t1[:, ow:bw], t2[:, ow:bw]
                                )
                                nc.vector.copy_predicated(
                                    ccell[:, ow:bw], m[:, : bw - ow],
                                    cn[:, : bw - ow],
                                )

                        th = tp.tile([128, CELLW], f32, tag="th")
                        nc.scalar.activation(th[:, ow:ce], ccell[:, ow:ce], AF.Tanh)

                        # h update: bulk in place, strip predicated
                        if ce > bw:
                            nc.vector.tensor_mul(
                                hcell[:, bw:ce], o_t[:, bw:ce], th[:, bw:ce]
                            )
                        if has_strip:
                            if d == "f":
                                m = mp.tile([128, 64], mybir.dt.uint8, tag="mask")
                                nc.vector.tensor_scalar(
                                    m[:, : bw - ow], lenb[:, lo:bk],
                                    float(t), None, ALU.is_gt,
                                )
                            hn = tp.tile([128, 64], dtm, tag="hn")
                            nc.vector.tensor_mul(
                                hn[:, : bw - ow], o_t[:, ow:bw], th[:, ow:bw]
                            )
                            nc.vector.copy_predicated(
                                hcell[:, ow:bw], m[:, : bw - ow], hn[:, : bw - ow]
                            )

            # ---- fc ----
            for ri, (s, e) in enumerate(cells):
                ce = e - s
                ps = pp.tile([128, 4, CELLW], f32, tag="ps")
                nc.tensor.matmul(
                    ps[:, 0, :ce], fcw[:, 0, :], hs["f"][ri][:, :ce],
                    start=True, stop=False,
                )
                nc.tensor.matmul(
                    ps[:, 0, :ce], fcw[:, 1, :], hs["b"][ri][:, :ce],
                    start=False, stop=True,
                )
                ot = op.tile([128, CELLW], f32, tag="o")
                nc.scalar.activation(
                    ot[:, :ce], ps[:, 0, :ce], AF.Identity, bias=fcb[:, 0:1]
                )
                nc.sync.dma_start(outT_d[:, s:e], ot[:, :ce])

    nc.compile()
    return nc


def get_program(T, R, r0_by_t, n_cores=N_CORES, mm_bf16=True):
    key = (T, R, tuple(r0_by_t), n_cores, mm_bf16)
    if key not in _PROGRAM_CACHE:
        _PROGRAM_CACHE.clear()
        _PROGRAM_CACHE[key] = build_program(T, R, r0_by_t, n_cores, mm_bf16)
    return _PROGRAM_CACHE[key]


# ---------------------------------------------------------------------------
# Device embedding pass (sequences -> [N, D] embeddings)
# ---------------------------------------------------------------------------

MM_BF16 = os.environ.get("NERANK_FP32", "0") != "1"


def bilstm_embed_device(seqs, lens, Wih_f, Whh_f, b_f, Wih_b, Whh_b, b_b,
                        fc_W, fc_b, n_cores=N_CORES, trace=None):
    """seqs: [N, L, I] f32, lens: [N] int.  Returns ([N, D] f32, exec_ns)."""
    import ml_dtypes

    from concourse.bass_utils import run_bass_kernel_spmd

    n = seqs.shape[0]
    order, lens_sorted, T, R, r0 = plan_sharding(lens, n_cores)

    nc = get_program(T, R, r0, n_cores, MM_BF16)
    dtm_np = ml_dtypes.bfloat16 if MM_BF16 else np.float32

    GPERM = [0, 1, 3, 2]  # reorder gate blocks (i,f,g,o) -> (i,f,o,g)

    def permg(w):
        w = np.asarray(w, np.float32)
        return w.reshape(-1, 4, H)[:, GPERM, :].reshape(w.shape[0], 4 * H)

    n_seq = seqs.shape[0]
    xT_aug = np.empty((T, I + 1, n_seq), dtm_np)
    xT_aug[:, :I, :] = seqs[:, :T, :].transpose(1, 2, 0).astype(dtm_np)
    xT_aug[:, I, :] = np.asarray(1.0, dtm_np)
    xT_sorted = xT_aug[:, :, order]
    del xT_aug

    def wih_aug(Wih, b):
        wp = permg(Wih)
        bp = permg(np.asarray(b, np.float32)[None, :])
        return np.ascontiguousarray(
            np.concatenate([wp, bp], axis=0).astype(dtm_np)
        )

    common = {
        "wih_f": wih_aug(Wih_f, b_f),
        "wih_b": wih_aug(Wih_b, b_b),
        "whh_f": np.ascontiguousarray(permg(Whh_f).astype(dtm_np)),
        "whh_b": np.ascontiguousarray(permg(Whh_b).astype(dtm_np)),
        "fcw": np.ascontiguousarray(np.asarray(fc_W).astype(dtm_np)),
        "fcb": np.ascontiguousarray(np.asarray(fc_b, np.float32).reshape(D, 1)),
    }
    in_maps = []
    for c in range(n_cores):
        lens_c = lens_sorted[c::n_cores].astype(np.float32)
        in_map = dict(common)
        in_map["xT"] = np.ascontiguousarray(xT_sorted[:, :, c::n_cores])
        in_map["lenb"] = np.ascontiguousarray(
            np.broadcast_to(lens_c[None, :], (128, R))
        )
        in_maps.append(in_map)
    del xT_sorted

    if trace is None:
        trace = os.environ.get("NERANK_TRACE", "0") == "1"
    res = run_bass_kernel_spmd(
        nc, in_maps, core_ids=list(range(n_cores)), trace=trace,
        trace_cores=list(range(n_cores)) if trace else None,
    )
    outs = [r["outT"] for r in res.results]  # each [D, R]

    emb_sorted = np.stack(outs, axis=0).transpose(2, 0, 1).reshape(n, D)
    emb = np.empty((n, D), np.float32)
    emb[order] = emb_sorted
    return emb, res.exec_time_ns


# ---------------------------------------------------------------------------
# Full problem
# ---------------------------------------------------------------------------

def _log_sigmoid64(x):
    return -np.logaddexp(0.0, -np.float64(x))


def kernel(**inputs):
    qu_seq = np.asarray(inputs["qu_seq"], np.float32)
    qn_seq = np.asarray(inputs["qn_seq"], np.float32).reshape(B * K, L, I)
    lens = np.concatenate(
        [
            np.asarray(inputs["qu_len"]).astype(np.int64),
            np.asarray(inputs["qn_len"]).astype(np.int64).reshape(-1),
        ]
    )
    seqs = np.concatenate([qu_seq, qn_seq], axis=0)
    del qu_seq, qn_seq

    emb, exec_ns = bilstm_embed_device(
        seqs, lens,
        np.asarray(inputs["Wih_f"]), np.asarray(inputs["Whh_f"]),
        np.asarray(inputs["b_f"]),
        np.asarray(inputs["Wih_b"]), np.asarray(inputs["Whh_b"]),
        np.asarray(inputs["b_b"]),
        np.asarray(inputs["fc_W"]), np.asarray(inputs["fc_b"]),
    )
    del seqs
    kernel.last_exec_time_ns = exec_ns
    kernel._last_emb = emb
    globals()["_last_emb"] = emb

    embed_qu = emb[:B]
    neg_q = emb[B:].reshape(B, K, D)

    ru_emb = np.asarray(inputs["ru_emb"])
    au_emb = np.asarray(inputs["au_emb"])
    rv_emb = np.asarray(inputs["rv_emb"])
    av_emb = np.asarray(inputs["av_emb"])
    idx = {k: np.asarray(inputs[k]).astype(np.int64) for k in
           ("ru_idx", "au_idx", "rv_idx", "av_idx", "rn_idx", "an_idx")}

    embed_u = ru_emb[idx["ru_idx"]] + au_emb[idx["au_idx"]] + embed_qu
    embed_v = rv_emb[idx["rv_idx"]] + av_emb[idx["av_idx"]] + embed_qu
    score = np.sum(np.float64(embed_u) * np.float64(embed_v))

    neg_v = av_emb[idx["an_idx"]] + rv_emb[idx["rn_idx"]] + neg_q
    neg_score = np.einsum("bkd,bd->", np.float64(neg_v), np.float64(embed_u))

    loss = _log_sigmoid64(score) + _log_sigmoid64(-neg_score)
    return np.array(loss, dtype=np.float32)


kernel.last_exec_time_ns = None


# revision 12
# speedup vs baseline: 4.2458x; 1.0065x over previous
"""NeRank loss kernel for 8 Trainium2 NeuronCores.

Strategy
--------
The dominant work is a shared-weight BiLSTM + fc applied to 12288 ragged
sequences (2048 positive + 2048*5 negative, each [L=64, I=300], lengths in
[1, 64]).  Everything after that (4 embedding gathers, elementwise adds, dot
products, log-sigmoid) reduces 12288x128 vectors to one scalar and is
negligible, so it runs on the host as the gather/unshard epilogue.

Device plan (SPMD, one program on 8 cores):
 * All 12288 sequences are sorted by length (ascending) and dealt round-robin
   to the 8 cores: global sorted rank i -> core i%8, column i//8.  Each core
   owns R=1536 columns, time-major transposed input xT[t, i, r].
 * Transposed LSTM layout: state h^T/c^T are [H=128 partitions, R free].
   Gates are computed as gates^T[4H, R] = Wih^T @ x_t^T + Whh^T @ h^T + b via
   TensorE matmuls (4 gate groups of 128 x (3 K-chunks of I + 1 K-chunk of H)).
 * Raggedness: since columns are sorted by length, the active columns at step
   t form a suffix [r0(t), R).  r0(t) is baked into the program (it is derived
   from the actual input lengths at kernel() time, min over cores so the same
   program is valid on all cores).  All DMA/matmul/vector work is restricted
   to the active suffix, so compute and memory scale with sum(len) rather than
   N*L.  Exact per-column semantics are enforced by a per-step mask
   (len > t) applied with copy_predicated on the h state only; the c state of
   finished columns may hold garbage, which is never observed.
 * Forward processes t = 0..T-1; backward processes t = T-1..0 (equivalent to
   the reference's reversed-sequence + reversed-mask formulation).  Both
   directions advance in the same program loop to keep all engines busy.
 * fc: out^T[D, R] = fc_W^T @ concat(h_f, h_b)^T + fc_b, then DMA out.

Host epilogue: un-permute embeddings, embedding-table gathers, dots,
log-sigmoid -> scalar loss.
"""

import os
from contextlib import ExitStack

import numpy as np

B, K, L, I, H, D = 2048, 5, 64, 300, 128, 128
N_CORES = 8
CELLW = 512  # R-tile width (one PSUM bank of fp32)

_PROGRAM_CACHE: dict = {}


# ---------------------------------------------------------------------------
# Host-side planning
# ---------------------------------------------------------------------------

def plan_sharding(lens: np.ndarray, n_cores: int):
    """Sort sequences by length, stripe across cores, compute active-suffix
    starts r0[t] (conservative min over cores: active columns at step t are
    exactly [r0_core(t), R) on every core with r0_core(t) >= r0[t], and
    columns < r0[t] are inactive on all cores)."""
    n = len(lens)
    assert n % n_cores == 0
    order = np.argsort(lens, kind="stable")
    lens_sorted = lens[order]
    T = int(lens_sorted[-1])
    R = n // n_cores
    r0 = [int(np.searchsorted(lens_sorted, t, side="right")) // n_cores
          for t in range(T)]
    return order, lens_sorted, T, R, r0


# ---------------------------------------------------------------------------
# Bass program builder
# ---------------------------------------------------------------------------

def build_program(T, R, r0_by_t, n_cores=N_CORES, mm_bf16=True):
    import concourse.bass as bass  # noqa: F401
    import concourse.mybir as mybir
    import concourse.tile as tile
    from concourse import bacc

    f32 = mybir.dt.float32
    dtm = mybir.dt.bfloat16 if mm_bf16 else f32  # matmul operand dtype
    AF = mybir.ActivationFunctionType
    ALU = mybir.AluOpType

    nc = bacc.Bacc(
        "TRN2",
        target_bir_lowering=False,
        debug=False,
        enable_asserts=False,
        num_devices=n_cores,
    )

    IA = I + 1  # x augmented with a constant-1 row; Wih row I holds the bias
    IPAD = 384  # x rows padded to 3*128 so each step loads with ONE dma_start
    xT_d = nc.dram_tensor("xT", [T, IPAD, R], dtm, kind="ExternalInput").ap()
    lenb_d = nc.dram_tensor("lenb", [128, R], f32, kind="ExternalInput").ap()
    wih_d = {
        "f": nc.dram_tensor("wih_f", [IA, 4 * H], dtm, kind="ExternalInput").ap(),
        "b": nc.dram_tensor("wih_b", [IA, 4 * H], dtm, kind="ExternalInput").ap(),
    }
    whh_d = {
        "f": nc.dram_tensor("whh_f", [H, 4 * H], dtm, kind="ExternalInput").ap(),
        "b": nc.dram_tensor("whh_b", [H, 4 * H], dtm, kind="ExternalInput").ap(),
    }
    fcw_d = nc.dram_tensor("fcw", [2 * H, D], dtm, kind="ExternalInput").ap()
    fcb_d = nc.dram_tensor("fcb", [D, 1], f32, kind="ExternalInput").ap()
    outT_d = nc.dram_tensor("outT", [D, R], f32, kind="ExternalOutput").ap()

    # I+1 split into K-chunks of <=128 partitions
    KCH = [(o, min(128, IA - o)) for o in range(0, IA, 128)]
    cells = [(s, min(s + CELLW, R)) for s in range(0, R, CELLW)]

    with tile.TileContext(nc) as tc:
        with ExitStack() as ctx:
            consts = ctx.enter_context(tc.tile_pool(name="consts", bufs=1))
            states = ctx.enter_context(tc.tile_pool(name="states", bufs=1))
            xp = ctx.enter_context(tc.tile_pool(name="xp", bufs=6))
            gp = ctx.enter_context(tc.tile_pool(name="gates", bufs=4))
            tp = ctx.enter_context(tc.tile_pool(name="tmps", bufs=4))
            mp = ctx.enter_context(tc.tile_pool(name="masks", bufs=4))
            op = ctx.enter_context(tc.tile_pool(name="outs", bufs=2))
            pp = ctx.enter_context(tc.tile_pool(name="psum", bufs=2, space="PSUM"))

            # ---- constants ----
            wih = {}
            for d in ("f", "b"):
                w = consts.tile([128, len(KCH), 4 * H], dtm, name=f"wih_{d}_sb")
                for ci, (o, kw) in enumerate(KCH):
                    nc.sync.dma_start(w[0:kw, ci, :], wih_d[d][o : o + kw, :])
                wih[d] = w
            whh = {}
            for d in ("f", "b"):
                w = consts.tile([128, 4 * H], dtm, name=f"whh_{d}_sb")
                nc.sync.dma_start(w[:], whh_d[d])
                whh[d] = w
            fcw = consts.tile([128, 2, D], dtm, name="fcw_sb")
            nc.sync.dma_start(fcw[:, 0, :], fcw_d[0:128, :])
            nc.sync.dma_start(fcw[:, 1, :], fcw_d[128:256, :])
            fcb = consts.tile([128, 1], f32, name="fcb_sb")
            nc.sync.dma_start(fcb[:], fcb_d)
            lenb = consts.tile([128, R], f32, name="lenb_sb")
            nc.sync.dma_start(lenb[:], lenb_d)

            # ---- states ----
            hs = {"f": [], "b": []}
            cs = {"f": [], "b": []}
            for d in ("f", "b"):
                for ri, (s, e) in enumerate(cells):
                    h = states.tile([128, e - s], dtm, name=f"h_{d}_{ri}")
                    c = states.tile([128, e - s], f32, name=f"c_{d}_{ri}")
                    nc.vector.memset(h[:], 0.0)
                    nc.vector.memset(c[:], 0.0)
                    hs[d].append(h)
                    cs[d].append(c)

            # gate column order (host-permuted): 0=i, 1=f, 2=o, 3=g
            # ---- recurrence ----
            for j in range(T):
                for d, t in (("f", j), ("b", T - 1 - j)):
                    a = r0_by_t[t]
                    if a >= R:
                        continue
                    for ri, (s, e) in enumerate(cells):
                        if a >= e:
                            continue
                        lo = max(s, (a // 32) * 32)
                        bk = max(s, min(a + 1, e))  # bulk start: active on ALL cores
                        ow = lo - s
                        bw = bk - s
                        ce = e - s
                        hcell = hs[d][ri]
                        ccell = cs[d][ri]
                        has_strip = bw > ow

                        x_ = xp.tile([128, 3, CELLW], dtm, tag="x")
                        nc.sync.dma_start(
                            x_[:, :, ow:ce],
                            xT_d[t].rearrange("(c p) r -> p c r", p=128)[:, :, lo:e],
                        )
                        xt = [x_[:, ci, :] for ci in range(len(KCH))]

                        ps = pp.tile([128, 4, CELLW], f32, tag="ps")
                        for ci, (o, kw) in enumerate(KCH):
                            for g in range(4):
                                nc.tensor.matmul(
                                    ps[:, g, ow:ce],
                                    wih[d][0:kw, ci, g * H : (g + 1) * H],
                                    xt[ci][0:kw, ow:ce],
                                    start=(ci == 0),
                                    stop=False,
                                )
                        for g in range(4):
                            nc.tensor.matmul(
                                ps[:, g, ow:ce],
                                whh[d][:, g * H : (g + 1) * H],
                                hcell[:, ow:ce],
                                start=False,
                                stop=True,
                            )

                        # activations: one sigmoid over (i,f,o), one tanh for g
                        sg = gp.tile([128, 3, CELLW], f32, tag="sg")
                        nc.scalar.activation(
                            sg[:, :, ow:ce], ps[:, 0:3, ow:ce], AF.Sigmoid
                        )
                        tg = gp.tile([128, CELLW], f32, tag="tg")
                        nc.scalar.activation(tg[:, ow:ce], ps[:, 3, ow:ce], AF.Tanh)

                        i_t = sg[:, 0, :]
                        f_t = sg[:, 1, :]
                        o_t = sg[:, 2, :]

                        t1 = tp.tile([128, CELLW], f32, tag="t1")
                        nc.vector.tensor_mul(
                            t1[:, ow:ce], f_t[:, ow:ce], ccell[:, ow:ce]
                        )
                        t2 = tp.tile([128, CELLW], f32, tag="t2")
                        nc.vector.tensor_mul(
                            t2[:, ow:ce], i_t[:, ow:ce], tg[:, ow:ce]
                        )

                        # c update: bulk columns (active on every core) in place
                        if d == "f":
                            # forward: inactive columns are finished; garbage c
                            # in the strip is never observed -> no predication.
                            nc.vector.tensor_add(
                                ccell[:, ow:ce], t1[:, ow:ce], t2[:, ow:ce]
                            )
                        else:
                            if ce > bw:
                                nc.vector.tensor_add(
                                    ccell[:, bw:ce], t1[:, bw:ce], t2[:, bw:ce]
                                )
                            if has_strip:
                                m = mp.tile(
                                    [128, 64], mybir.dt.uint8, tag="mask"
                                )
                                nc.vector.tensor_scalar(
                                    m[:, : bw - ow], lenb[:, lo:bk],
                                    float(t), None, ALU.is_gt,
                                )
                                cn = tp.tile([128, 64], f32, tag="cn")
                                nc.vector.tensor_add(
                                    cn[:, : bw - ow], t1[:, ow:bw], t2[:, ow:bw]
                                )
                                nc.vector.copy_predicated(
                                    ccell[:, ow:bw], m[:, : bw - ow],
                                    cn[:, : bw - ow],
                                )

                        th = tp.tile([128, CELLW], f32, tag="th")
                        nc.scalar.activation(th[:, ow:ce], ccell[:, ow:ce], AF.Tanh)

                        # h update: bulk in place, strip predicated
                        if ce > bw:
                            nc.vector.tensor_mul(
                                hcell[:, bw:ce], o_t[:, bw:ce], th[:, bw:ce]
                            )
                        if has_strip:
                            if d == "f":
                                m = mp.tile([128, 64], mybir.dt.uint8, tag="mask")
                                nc.vector.tensor_scalar(
                                    m[:, : bw - ow], lenb[:, lo:bk],
                                    float(t), None, ALU.is_gt,
                                )
                            hn = tp.tile([128, 64], dtm, tag="hn")
                            nc.vector.tensor_mul(
                                hn[:, : bw - ow], o_t[:, ow:bw], th[:, ow:bw]
                            )
                            nc.vector.copy_predicated(
                                hcell[:, ow:bw], m[:, : bw - ow], hn[:, : bw - ow]
                            )

            # ---- fc ----
            for ri, (s, e) in enumerate(cells):
                ce = e - s
                ps = pp.tile([128, 4, CELLW], f32, tag="ps")
                nc.tensor.matmul(
                    ps[:, 0, :ce], fcw[:, 0, :], hs["f"][ri][:, :ce],
                    start=True, stop=False,
                )
                nc.tensor.matmul(
                    ps[:, 0, :ce], fcw[:, 1, :], hs["b"][ri][:, :ce],
                    start=False, stop=True,
                )
                ot = op.tile([128, CELLW], f32, tag="o")
                nc.scalar.activation(
                    ot[:, :ce], ps[:, 0, :ce], AF.Identity, bias=fcb[:, 0:1]
                )
                nc.sync.dma_start(outT_d[:, s:e], ot[:, :ce])

    nc.compile()
    return nc


def get_program(T, R, r0_by_t, n_cores=N_CORES, mm_bf16=True):
    key = (T, R, tuple(r0_by_t), n_cores, mm_bf16)
    if key not in _PROGRAM_CACHE:
        _PROGRAM_CACHE.clear()
        _PROGRAM_CACHE[key] = build_program(T, R, r0_by_t, n_cores, mm_bf16)
    return _PROGRAM_CACHE[key]


# ---------------------------------------------------------------------------
# Device embedding pass (sequences -> [N, D] embeddings)
# ---------------------------------------------------------------------------

MM_BF16 = os.environ.get("NERANK_FP32", "0") != "1"


def bilstm_embed_device(seqs, lens, Wih_f, Whh_f, b_f, Wih_b, Whh_b, b_b,
                        fc_W, fc_b, n_cores=N_CORES, trace=None):
    """seqs: [N, L, I] f32, lens: [N] int.  Returns ([N, D] f32, exec_ns)."""
    import ml_dtypes

    from concourse.bass_utils import run_bass_kernel_spmd

    n = seqs.shape[0]
    order, lens_sorted, T, R, r0 = plan_sharding(lens, n_cores)

    nc = get_program(T, R, r0, n_cores, MM_BF16)
    dtm_np = ml_dtypes.bfloat16 if MM_BF16 else np.float32

    GPERM = [0, 1, 3, 2]  # reorder gate blocks (i,f,g,o) -> (i,f,o,g)

    def permg(w):
        w = np.asarray(w, np.float32)
        return w.reshape(-1, 4, H)[:, GPERM, :].reshape(w.shape[0], 4 * H)

    n_seq = seqs.shape[0]
    xT_aug = np.zeros((T, 384, n_seq), dtm_np)
    xT_aug[:, :I, :] = seqs[:, :T, :].transpose(1, 2, 0).astype(dtm_np)
    xT_aug[:, I, :] = np.asarray(1.0, dtm_np)
    xT_sorted = xT_aug[:, :, order]
    del xT_aug

    def wih_aug(Wih, b):
        wp = permg(Wih)
        bp = permg(np.asarray(b, np.float32)[None, :])
        return np.ascontiguousarray(
            np.concatenate([wp, bp], axis=0).astype(dtm_np)
        )

    common = {
        "wih_f": wih_aug(Wih_f, b_f),
        "wih_b": wih_aug(Wih_b, b_b),
        "whh_f": np.ascontiguousarray(permg(Whh_f).astype(dtm_np)),
        "whh_b": np.ascontiguousarray(permg(Whh_b).astype(dtm_np)),
        "fcw": np.ascontiguousarray(np.asarray(fc_W).astype(dtm_np)),
        "fcb": np.ascontiguousarray(np.asarray(fc_b, np.float32).reshape(D, 1)),
    }
    in_maps = []
    for c in range(n_cores):
        lens_c = lens_sorted[c::n_cores].astype(np.float32)
        in_map = dict(common)
        in_map["xT"] = np.ascontiguousarray(xT_sorted[:, :, c::n_cores])
        in_map["lenb"] = np.ascontiguousarray(
            np.broadcast_to(lens_c[None, :], (128, R))
        )
        in_maps.append(in_map)
    del xT_sorted

    if trace is None:
        trace = os.environ.get("NERANK_TRACE", "0") == "1"
    res = run_bass_kernel_spmd(
        nc, in_maps, core_ids=list(range(n_cores)), trace=trace,
        trace_cores=list(range(n_cores)) if trace else None,
    )
    outs = [r["outT"] for r in res.results]  # each [D, R]

    emb_sorted = np.stack(outs, axis=0).transpose(2, 0, 1).reshape(n, D)
    emb = np.empty((n, D), np.float32)
    emb[order] = emb_sorted
    return emb, res.exec_time_ns


# ---------------------------------------------------------------------------
# Full problem
# ---------------------------------------------------------------------------

def _log_sigmoid64(x):
    return -np.logaddexp(0.0, -np.float64(x))


def kernel(**inputs):
    qu_seq = np.asarray(inputs["qu_seq"], np.float32)
    qn_seq = np.asarray(inputs["qn_seq"], np.float32).reshape(B * K, L, I)
    lens = np.concatenate(
        [
            np.asarray(inputs["qu_len"]).astype(np.int64),
            np.asarray(inputs["qn_len"]).astype(np.int64).reshape(-1),
        ]
    )
    seqs = np.concatenate([qu_seq, qn_seq], axis=0)
    del qu_seq, qn_seq

    emb, exec_ns = bilstm_embed_device(
        seqs, lens,
        np.asarray(inputs["Wih_f"]), np.asarray(inputs["Whh_f"]),
        np.asarray(inputs["b_f"]),
        np.asarray(inputs["Wih_b"]), np.asarray(inputs["Whh_b"]),
        np.asarray(inputs["b_b"]),
        np.asarray(inputs["fc_W"]), np.asarray(inputs["fc_b"]),
    )
    del seqs
    kernel.last_exec_time_ns = exec_ns
    kernel._last_emb = emb
    globals()["_last_emb"] = emb

    embed_qu = emb[:B]
    neg_q = emb[B:].reshape(B, K, D)

    ru_emb = np.asarray(inputs["ru_emb"])
    au_emb = np.asarray(inputs["au_emb"])
    rv_emb = np.asarray(inputs["rv_emb"])
    av_emb = np.asarray(inputs["av_emb"])
    idx = {k: np.asarray(inputs[k]).astype(np.int64) for k in
           ("ru_idx", "au_idx", "rv_idx", "av_idx", "rn_idx", "an_idx")}

    embed_u = ru_emb[idx["ru_idx"]] + au_emb[idx["au_idx"]] + embed_qu
    embed_v = rv_emb[idx["rv_idx"]] + av_emb[idx["av_idx"]] + embed_qu
    score = np.sum(np.float64(embed_u) * np.float64(embed_v))

    neg_v = av_emb[idx["an_idx"]] + rv_emb[idx["rn_idx"]] + neg_q
    neg_score = np.einsum("bkd,bd->", np.float64(neg_v), np.float64(embed_u))

    loss = _log_sigmoid64(score) + _log_sigmoid64(-neg_score)
    return np.array(loss, dtype=np.float32)


kernel.last_exec_time_ns = None
